# revision 4
# baseline (speedup 1.0000x reference)
"""Bass/Tile TRN2 kernel: adaptive min 2D pooling (8x8 grid) of [B,512,512] f32.

Full input [128, 512, 512] f32 -> output [128, 64] f32.
Data parallel over 8 NeuronCores: 16 matrices per core.

Per-core algorithm (x_local [16, 512, 512] -> y_local [16, 64]):
  1. For each matrix m: DMA [512,512] into SBUF as [128, 2048] with
     partition p = row within a 128-row quarter, free = (q, c):
     row = q*128 + p. Each partition line is 4 chunks of 2KB contiguous
     HBM -> near line-rate DMA.
  2. DVE reduce_min over the innermost 64-column groups:
     [128, (q gc c=64)] -> acc[:, m*32 + (q*8+gc)]  (min over c).
     acc is [128, 512] with free = (m, q, gc), partition = row-in-quarter.
  3. Cross-partition min (over the 128 rows-in-quarter = 2 bands x 64 rows)
     cannot run on DVE -> PE-transpose each 128-col block of acc into PSUM
     (exact data movement), then DVE reduce_min over the row halves:
     accT_k [128=(m' q gc), (b=2, r=64)] -> res[:, 2k+b].
  4. Second PE transpose turns res [128, (k b)] into resT [(k b), (mp q gc)]
     (via PSUM + DVE copy), so the 8 output DMAs read contiguous 512B SBUF
     lines and write 32B-contiguous runs into y [16, 64].
"""

import threading

import numpy as np

B, N, M = 128, 512, 512
GRID = 8
NCORES = 8
BL = B // NCORES  # 16 matrices per core

_lock = threading.RLock()
_cache: dict = {}


def _build(n_iters: int = 1, bufs: int = 6):
    import concourse.bacc as bacc
    import concourse.mybir as mybir
    import concourse.tile as tile

    f32 = mybir.dt.float32

    nc = bacc.Bacc("TRN2", target_bir_lowering=False, debug=False)
    x = nc.dram_tensor("x", [BL, N, M], f32, kind="ExternalInput").ap()
    y = nc.dram_tensor("y", [BL, GRID * GRID], f32, kind="ExternalOutput").ap()

    with tile.TileContext(nc) as tc:
        with (
            tc.tile_pool(name="inp", bufs=bufs) as inp,
            tc.tile_pool(name="accp", bufs=2) as accp,
            tc.tile_pool(name="resp", bufs=2) as resp,
            tc.tile_pool(name="rtp", bufs=2) as rtp,
            tc.tile_pool(name="idp", bufs=1) as idp,
            tc.tile_pool(name="psw", bufs=2, space="PSUM") as psw,
            tc.tile_pool(name="ps2", bufs=2, space="PSUM") as ps2,
        ):
            # identity matrix for the PE transpose
            ones = idp.tile([128, 128], f32)
            ident = idp.tile([128, 128], f32)
            nc.gpsimd.memset(ones[:], 1.0)
            nc.gpsimd.affine_select(
                ident[:],
                ones[:],
                pattern=[[-1, 128]],
                compare_op=mybir.AluOpType.is_equal,
                fill=0.0,
                base=0,
                channel_multiplier=1,
            )

            # Deferred-output software pipeline: iteration i's 8 y-write DMAs
            # are issued AFTER iteration i+1's x-load DMAs in ring program
            # order. HWDGE rings are FIFO per issuing engine, so a y-write
            # whose data isn't ready stalls every later x-load on that ring;
            # deferring by one iteration removes the per-iteration stage-2
            # latency (~3us) from the steady-state DMA stream.
            def issue_y(resT):
                for k in range(4):
                    for b in range(2):
                        eng = nc.sync if (2 * k + b) % 2 == 0 else nc.scalar
                        eng.dma_start(
                            y[4 * k : 4 * (k + 1)].rearrange(
                                "mp (q b gc) -> mp q b gc", q=4, b=2
                            )[:, :, b, :],
                            resT[2 * k + b : 2 * k + b + 1, :],
                        )

            pending = None
            for _ in range(n_iters):  # n_iters>1 only for benchmarking
                acc = accp.tile([128, 512], f32)
                res = resp.tile([128, 8], f32)

                # stage 1: per-matrix load + min over column groups.
                # Alternate the two HWDGE rings (SP / ACT) — a single ring
                # serializes with ~1us bubbles between DMAs (300 GB/s); the
                # two rings together reach the HBM roofline (~373 GB/s).
                # The last matrix is split into 4 quarter DMAs/reduces so the
                # kernel tail only waits on a 256KB transfer + small reduce
                # (-10us measured). Interleaving stage 2 into this stream was
                # measured SLOWER on HW (+12-30us; PE/PSUM traffic and scatter
                # DMAs stall the FIFO DMA rings mid-stream), so stage 2 stays
                # at the end.
                for m in range(BL):
                    t = inp.tile([128, 4 * M], f32)
                    if m == BL - 1:
                        for q in range(4):
                            eng = nc.sync if q % 2 == 0 else nc.scalar
                            eng.dma_start(
                                t[:, q * M : (q + 1) * M],
                                x[m, q * 128 : (q + 1) * 128, :],
                            )
                            nc.vector.tensor_reduce(
                                acc[:, m * 32 + q * 8 : m * 32 + (q + 1) * 8],
                                t[:, q * M : (q + 1) * M].rearrange(
                                    "p (g c) -> p g c", c=M // GRID
                                ),
                                axis=mybir.AxisListType.X,
                                op=mybir.AluOpType.min,
                            )
                    else:
                        eng = nc.sync if m % 2 == 0 else nc.scalar
                        eng.dma_start(
                            t[:].rearrange("p (q c) -> p q c", q=4),
                            x[m].rearrange("(q p) c -> p q c", p=128),
                        )
                        nc.vector.tensor_reduce(
                            acc[:, m * 32 : (m + 1) * 32],
                            t[:].rearrange("p (g c) -> p g c", c=M // GRID),
                            axis=mybir.AxisListType.X,
                            op=mybir.AluOpType.min,
                        )

                if pending is not None:
                    issue_y(pending)

                # stage 2: cross-partition min via PE transpose + free-dim
                # reduce over the row halves (bands). All 4 transposes land
                # in one PSUM-bank tile so a single fused reduce replaces 4.
                pt_all = psw.tile([128, 512], f32)
                for k in range(4):
                    nc.tensor.transpose(
                        pt_all[:, k * 128 : (k + 1) * 128],
                        acc[:, k * 128 : (k + 1) * 128],
                        ident[:],
                    )
                nc.vector.tensor_reduce(
                    res[:],
                    pt_all[:].rearrange("p (g r) -> p g r", r=64),
                    axis=mybir.AxisListType.X,
                    op=mybir.AluOpType.min,
                )

                # Second PE transpose: res [128,(k b)] -> resT [(k b), (mp q gc)]
                # so each output DMA reads one 512B-contiguous SBUF line and
                # writes 16 x 32B runs, instead of 128 x 4B scattered writes
                # (-9us/iter measured head-to-head).
                pt2 = ps2.tile([128, 128], f32)
                nc.tensor.transpose(pt2[0:8, :], res[:], ident[:])
                resT = rtp.tile([128, 128], f32)
                # DVE does the PSUM->SBUF copy: ACT must stay a pure DMA ring
                # (an ACT compute op waiting on stage 2 would stall the next
                # iteration's x-loads queued behind it).
                nc.vector.tensor_scalar_add(resT[0:8, :], pt2[0:8, :], 0.0)
                # resT[(k b), (mp q gc)] -> y[4k+mp, (2q+b)*8+gc], deferred
                pending = resT

            issue_y(pending)

    nc.compile()
    return nc


def _get_nc():
    with _lock:
        if "nc" not in _cache:
            _cache["nc"] = _build()
        return _cache["nc"]


def _get_runner():
    """Build the shard_map-jitted 8-core runner ONCE and reuse it across
    kernel() calls (run_bass_kernel_spmd re-jits per call, ~seconds of host
    overhead). Mirrors bass2jax.run_bass_via_pjrt's multi-core wiring."""
    if "runner" in _cache:
        return _cache["runner"]

    import jax
    from jax.sharding import Mesh, PartitionSpec
    from jax.experimental.shard_map import shard_map

    from concourse import bass2jax, mybir

    nc = _get_nc()
    bass2jax.install_neuronx_cc_hook()

    partition_name = nc.partition_id_tensor.name if nc.partition_id_tensor else None
    in_names, out_names, out_avals = [], [], []
    for alloc in nc.m.functions[0].allocations:
        if not isinstance(alloc, mybir.MemoryLocationSet):
            continue
        name = alloc.memorylocations[0].name
        if alloc.kind == "ExternalInput":
            if name != partition_name:
                in_names.append(name)
        elif alloc.kind == "ExternalOutput":
            out_names.append(name)
            out_avals.append(
                jax.core.ShapedArray(
                    tuple(alloc.tensor_shape), mybir.dt.np(alloc.dtype)
                )
            )
    assert in_names == ["x"] and out_names == ["y"]
    all_in_names = list(in_names) + list(out_names)
    if partition_name is not None:
        all_in_names.append(partition_name)

    def _body(*args):
        operands = list(args)
        if partition_name is not None:
            operands.append(bass2jax.partition_id_tensor())
        outs = bass2jax._bass_exec_p.bind(
            *operands,
            out_avals=tuple(out_avals),
            in_names=tuple(all_in_names),
            out_names=tuple(out_names),
            lowering_input_output_aliases=(),
            sim_require_finite=True,
            sim_require_nnan=True,
            nc=nc,
        )
        return tuple(outs)

    devices = jax.devices()[:NCORES]
    mesh = Mesh(np.asarray(devices), ("core",))
    sharded = jax.jit(
        shard_map(
            _body,
            mesh=mesh,
            in_specs=(PartitionSpec("core"),) * 2,
            out_specs=(PartitionSpec("core"),),
            check_rep=False,
        ),
        donate_argnums=(1,),
        keep_unused=True,
    )
    _cache["runner"] = sharded
    return sharded


def _kernel_fallback(xs: np.ndarray) -> np.ndarray:
    from concourse.bass_utils import run_bass_kernel_spmd

    nc = _get_nc()
    in_maps = [{"x": xs[i * BL : (i + 1) * BL]} for i in range(NCORES)]
    r = run_bass_kernel_spmd(nc, in_maps, list(range(NCORES)))
    return np.concatenate([r.results[i]["y"] for i in range(NCORES)], axis=0)


def kernel(sim_matrices: np.ndarray) -> np.ndarray:
    xs = np.ascontiguousarray(sim_matrices, dtype=np.float32)
    assert xs.shape == (B, N, M), xs.shape
    with _lock:
        try:
            runner = _get_runner()
            zeros = np.zeros((B, GRID * GRID), np.float32)
            (y_global,) = runner(xs, zeros)
            return np.asarray(y_global)
        except Exception:
            return _kernel_fallback(xs)



# revision 5
# speedup vs baseline: 1.4015x; 1.4015x over previous
"""Bass/Tile TRN2 kernel: adaptive min 2D pooling (8x8 grid) of [B,512,512] f32.

Full input [128, 512, 512] f32 -> output [128, 64] f32.
Data parallel over 8 NeuronCores: 16 matrices per core.

Per-core algorithm (x_local [16, 512, 512] -> y_local [16, 64]):
  1. For each matrix m: one flat DMA [128, 2048] (partition p = rows
     4p..4p+3, i.e. one 8KB-contiguous HBM run per partition), alternating
     the two HWDGE rings (SP / ACT). Both rings together saturate the
     per-NeuronCore HBM limit (~350 GB/s measured, ~358 GB/s spec).
  2. One DVE tensor_reduce per matrix, axis=XY over (a=4 rows, c=64 cols)
     per col-group: t[p, (a g c)] -> acc[p, m*8+g]. acc is [128, (m g)].
  3. Cross-partition min: single PE transpose acc -> accT[(m g), p] in
     PSUM, where p = (band b=8, quad qd=16) since row = 4p. DVE reduce X
     over qd -> res[(m g), b].
  4. PE transpose res -> pt2[b, (m g)] (PSUM), DVE copy to SBUF, then ONE
     SWDGE (gpsimd) DMA scatters y[m, b*8+g]. The y-write goes via SWDGE
     so the HWDGE rings carry nothing but x-loads: HWDGE rings are FIFO
     per issuing engine, so any y-write queued on them would stall the
     next iteration's x-load stream until stage 2 finishes (~3 us/iter,
     measured on v1/v2 vs v3 loop-slope A/B).

Measured (hardware-loop slope, 2016-iteration delta, median of rounds):
  v1 (2KB-desc loads + HWDGE y-writes at end): 51.8 us/iter
  v3 (this kernel):                            48.3-48.7 us/iter
  pure-DMA floor (same loads, no compute):     48.0-48.6 us/iter
i.e. v3 sits at the per-core HBM roofline; 16.78 MB / 48.3 us = 347 GB/s.
"""

import threading

import numpy as np

B, N, M = 128, 512, 512
GRID = 8
NCORES = 8
BL = B // NCORES  # 16 matrices per core

_lock = threading.RLock()
_cache: dict = {}


def _emit_prologue(nc, tc, mybir, idp):
    f32 = mybir.dt.float32
    ones = idp.tile([128, 128], f32)
    ident = idp.tile([128, 128], f32)
    nc.gpsimd.memset(ones[:], 1.0)
    nc.gpsimd.affine_select(
        ident[:],
        ones[:],
        pattern=[[-1, 128]],
        compare_op=mybir.AluOpType.is_equal,
        fill=0.0,
        base=0,
        channel_multiplier=1,
    )
    return ident


def _emit_iteration(nc, mybir, x, y, ident, inp, accp, resp, rtp, psw, ps2):
    f32 = mybir.dt.float32

    # stage 1: flat loads + fused (rows x cols) min per col-group
    acc = accp.tile([128, 128], f32)
    for m in range(BL):
        t = inp.tile([128, 4 * M], f32)
        eng = nc.sync if m % 2 == 0 else nc.scalar
        eng.dma_start(
            t[:],
            x[m].rearrange("(p a) c -> p (a c)", p=128),
        )
        nc.vector.tensor_reduce(
            acc[:, m * 8 : (m + 1) * 8],
            t[:].rearrange("p (a g c) -> p g a c", a=4, c=M // GRID),
            axis=mybir.AxisListType.XY,
            op=mybir.AluOpType.min,
        )

    # stage 2: cross-partition min via one PE transpose + banded reduce
    pt_all = psw.tile([128, 128], f32)
    nc.tensor.transpose(pt_all[:], acc[:], ident[:])
    res = resp.tile([128, 8], f32)
    nc.vector.tensor_reduce(
        res[:],
        pt_all[:].rearrange("p (b qd) -> p b qd", qd=16),
        axis=mybir.AxisListType.X,
        op=mybir.AluOpType.min,
    )

    # res[(m g), b] -> pt2[b, (m g)] -> resT (SBUF; DVE does the PSUM
    # read so ACT stays a pure DMA ring)
    pt2 = ps2.tile([128, 128], f32)
    nc.tensor.transpose(pt2[0:8, :], res[:], ident[:])
    resT = rtp.tile([128, 128], f32)
    nc.vector.tensor_scalar_add(resT[0:8, :], pt2[0:8, :], 0.0)

    # one SWDGE DMA: resT[b, (m g)] -> y[m, b*8+g] (off the HWDGE rings)
    nc.gpsimd.dma_start(
        y[:].rearrange("m (b g) -> b m g", b=8),
        resT[0:8, :].rearrange("b (m g) -> b m g", g=8),
    )


def _make_nc():
    import concourse.bacc as bacc
    import concourse.mybir as mybir

    f32 = mybir.dt.float32
    nc = bacc.Bacc("TRN2", target_bir_lowering=False, debug=False)
    x = nc.dram_tensor("x", [BL, N, M], f32, kind="ExternalInput").ap()
    y = nc.dram_tensor("y", [BL, GRID * GRID], f32, kind="ExternalOutput").ap()
    return nc, mybir, x, y


def _build(n_iters: int = 1, bufs: int = 6):
    """Python-unrolled build (n_iters=1 is what kernel() runs)."""
    import concourse.tile as tile

    nc, mybir, x, y = _make_nc()
    with tile.TileContext(nc) as tc:
        with (
            tc.tile_pool(name="inp", bufs=bufs) as inp,
            tc.tile_pool(name="accp", bufs=2) as accp,
            tc.tile_pool(name="resp", bufs=2) as resp,
            tc.tile_pool(name="rtp", bufs=2) as rtp,
            tc.tile_pool(name="idp", bufs=1) as idp,
            tc.tile_pool(name="psw", bufs=2, space="PSUM") as psw,
            tc.tile_pool(name="ps2", bufs=2, space="PSUM") as ps2,
        ):
            ident = _emit_prologue(nc, tc, mybir, idp)
            for _ in range(n_iters):
                _emit_iteration(
                    nc, mybir, x, y, ident, inp, accp, resp, rtp, psw, ps2
                )
    nc.compile()
    return nc


def _build_loop(n_bodies: int, unroll: int = 16, bufs: int = 6):
    """Hardware-looped build (for low-noise timing): body = `unroll`
    iterations inside tc.For_i, so device time scales without NEFF bloat."""
    import concourse.tile as tile

    nc, mybir, x, y = _make_nc()
    with tile.TileContext(nc) as tc:
        with (
            tc.tile_pool(name="inp", bufs=bufs) as inp,
            tc.tile_pool(name="accp", bufs=2) as accp,
            tc.tile_pool(name="resp", bufs=2) as resp,
            tc.tile_pool(name="rtp", bufs=2) as rtp,
            tc.tile_pool(name="idp", bufs=1) as idp,
            tc.tile_pool(name="psw", bufs=2, space="PSUM") as psw,
            tc.tile_pool(name="ps2", bufs=2, space="PSUM") as ps2,
        ):
            ident = _emit_prologue(nc, tc, mybir, idp)
            with tc.For_i(0, n_bodies, 1):
                for _ in range(unroll):
                    _emit_iteration(
                        nc, mybir, x, y, ident, inp, accp, resp, rtp, psw, ps2
                    )
    nc.compile()
    return nc


def _get_nc():
    with _lock:
        if "nc" not in _cache:
            _cache["nc"] = _build()
        return _cache["nc"]


def _get_runner():
    """Build the shard_map-jitted 8-core runner ONCE and reuse it across
    kernel() calls (run_bass_kernel_spmd re-jits per call, ~seconds of host
    overhead). Mirrors bass2jax.run_bass_via_pjrt's multi-core wiring."""
    if "runner" in _cache:
        return _cache["runner"]

    import jax
    from jax.sharding import Mesh, PartitionSpec
    from jax.experimental.shard_map import shard_map

    from concourse import bass2jax, mybir

    nc = _get_nc()
    bass2jax.install_neuronx_cc_hook()

    partition_name = nc.partition_id_tensor.name if nc.partition_id_tensor else None
    in_names, out_names, out_avals = [], [], []
    for alloc in nc.m.functions[0].allocations:
        if not isinstance(alloc, mybir.MemoryLocationSet):
            continue
        name = alloc.memorylocations[0].name
        if alloc.kind == "ExternalInput":
            if name != partition_name:
                in_names.append(name)
        elif alloc.kind == "ExternalOutput":
            out_names.append(name)
            out_avals.append(
                jax.core.ShapedArray(
                    tuple(alloc.tensor_shape), mybir.dt.np(alloc.dtype)
                )
            )
    assert in_names == ["x"] and out_names == ["y"]
    all_in_names = list(in_names) + list(out_names)
    if partition_name is not None:
        all_in_names.append(partition_name)

    def _body(*args):
        operands = list(args)
        if partition_name is not None:
            operands.append(bass2jax.partition_id_tensor())
        outs = bass2jax._bass_exec_p.bind(
            *operands,
            out_avals=tuple(out_avals),
            in_names=tuple(all_in_names),
            out_names=tuple(out_names),
            lowering_input_output_aliases=(),
            sim_require_finite=True,
            sim_require_nnan=True,
            nc=nc,
        )
        return tuple(outs)

    devices = jax.devices()[:NCORES]
    mesh = Mesh(np.asarray(devices), ("core",))
    sharded = jax.jit(
        shard_map(
            _body,
            mesh=mesh,
            in_specs=(PartitionSpec("core"),) * 2,
            out_specs=(PartitionSpec("core"),),
            check_rep=False,
        ),
        donate_argnums=(1,),
        keep_unused=True,
    )
    _cache["runner"] = sharded
    return sharded


def _kernel_fallback(xs: np.ndarray) -> np.ndarray:
    from concourse.bass_utils import run_bass_kernel_spmd

    nc = _get_nc()
    in_maps = [{"x": xs[i * BL : (i + 1) * BL]} for i in range(NCORES)]
    r = run_bass_kernel_spmd(nc, in_maps, list(range(NCORES)))
    return np.concatenate([r.results[i]["y"] for i in range(NCORES)], axis=0)


def kernel(sim_matrices: np.ndarray) -> np.ndarray:
    xs = np.ascontiguousarray(sim_matrices, dtype=np.float32)
    assert xs.shape == (B, N, M), xs.shape
    with _lock:
        try:
            runner = _get_runner()
            zeros = np.zeros((B, GRID * GRID), np.float32)
            (y_global,) = runner(xs, zeros)
            return np.asarray(y_global)
        except Exception:
            return _kernel_fallback(xs)


# revision 19
# speedup vs baseline: 1.4319x; 1.0217x over previous
"""Bass/Tile TRN2 kernel: adaptive min 2D pooling (8x8 grid) of [B,512,512] f32.

Full input [128, 512, 512] f32 -> output [128, 64] f32.
Data parallel over 8 NeuronCores: 16 matrices per core.

Per-core algorithm (x_local [16, 512, 512] -> y_local [16, 64]):
  1. For each matrix m: one flat DMA [128, 2048] (partition p = rows
     4p..4p+3, i.e. one 8KB-contiguous HBM run per partition), alternating
     the two HWDGE rings (SP / ACT). Both rings together saturate the
     per-NeuronCore HBM limit (~350 GB/s measured, ~358 GB/s spec).
  2. One DVE tensor_reduce per matrix, axis=XY over (a=4 rows, c=64 cols)
     per col-group: t[p, (a g c)] -> acc[p, m*8+g]. acc is [128, (m g)].
  3. Cross-partition min: single PE transpose acc -> accT[(m g), p] in
     PSUM, where p = (band b=8, quad qd=16) since row = 4p. DVE reduce X
     over qd -> res[(m g), b].
  4. PE transpose res -> pt2[b, (m g)] (PSUM), DVE copy to SBUF, then ONE
     SWDGE (gpsimd) DMA scatters y[m, b*8+g]. The y-write goes via SWDGE
     so the HWDGE rings carry nothing but x-loads: HWDGE rings are FIFO
     per issuing engine, so any y-write queued on them would stall the
     next iteration's x-load stream until stage 2 finishes (~3 us/iter,
     measured on v1/v2 vs v3 loop-slope A/B).

Measured (hardware-loop slope, 2016-iteration delta, median of rounds;
machine drifts ~±1 us between sessions):
  v1 (2KB-desc loads + HWDGE y-writes at end): 51.8 us/iter
  v3 (this kernel):                            47.6-48.7 us/iter
  pure-DMA floor (same loads, no compute):     47.7-48.6 us/iter
i.e. v3 sits at the per-core HBM roofline; 16.78 MB / 48 us = 350 GB/s.
Also A/B'd at the roofline (no change): 16KB/32KB-per-partition descriptors
(pair/quad loads), 3rd DMA stream via SWDGE (2 us WORSE), For_i unroll=32 +
branch-prefetch hints (1.3 us WORSE than unroll=16, measurement-only).
"""

import threading

import numpy as np

B, N, M = 128, 512, 512
GRID = 8
NCORES = 8
BL = B // NCORES  # 16 matrices per core

_lock = threading.RLock()
_cache: dict = {}


def _emit_prologue(nc, tc, mybir, idp):
    f32 = mybir.dt.float32
    ones = idp.tile([128, 128], f32)
    ident = idp.tile([128, 128], f32)
    nc.gpsimd.memset(ones[:], 1.0)
    nc.gpsimd.affine_select(
        ident[:],
        ones[:],
        pattern=[[-1, 128]],
        compare_op=mybir.AluOpType.is_equal,
        fill=0.0,
        base=0,
        channel_multiplier=1,
    )
    return ident


def _emit_iteration(nc, mybir, x, y, ident, inp, accp, resp, rtp, psw, ps2):
    f32 = mybir.dt.float32

    # stage 1: flat loads + fused (rows x cols) min per col-group
    acc = accp.tile([128, 128], f32)
    for m in range(BL):
        t = inp.tile([128, 4 * M], f32)
        eng = nc.sync if m % 2 == 0 else nc.scalar
        eng.dma_start(
            t[:],
            x[m].rearrange("(p a) c -> p (a c)", p=128),
        )
        nc.vector.tensor_reduce(
            acc[:, m * 8 : (m + 1) * 8],
            t[:].rearrange("p (a g c) -> p g a c", a=4, c=M // GRID),
            axis=mybir.AxisListType.XY,
            op=mybir.AluOpType.min,
        )

    # stage 2: cross-partition min via one PE transpose + banded reduce
    pt_all = psw.tile([128, 128], f32)
    nc.tensor.transpose(pt_all[:], acc[:], ident[:])
    res = resp.tile([128, 8], f32)
    nc.vector.tensor_reduce(
        res[:],
        pt_all[:].rearrange("p (b qd) -> p b qd", qd=16),
        axis=mybir.AxisListType.X,
        op=mybir.AluOpType.min,
    )

    # res[(m g), b] -> pt2[b, (m g)] -> resT (SBUF; DVE does the PSUM
    # read so ACT stays a pure DMA ring)
    pt2 = ps2.tile([128, 128], f32)
    nc.tensor.transpose(pt2[0:8, :], res[:], ident[:])
    resT = rtp.tile([128, 128], f32)
    nc.vector.tensor_scalar_add(resT[0:8, :], pt2[0:8, :], 0.0)

    # one SWDGE DMA: resT[b, (m g)] -> y[m, b*8+g] (off the HWDGE rings)
    nc.gpsimd.dma_start(
        y[:].rearrange("m (b g) -> b m g", b=8),
        resT[0:8, :].rearrange("b (m g) -> b m g", g=8),
    )


def _emit_iteration_tail(
    nc, mybir, x, y, ident, inp, accp, resp, rtp, psw, ps2,
    tail_split=True, y_hwdge=True,
):
    """Latency-optimized final iteration (what a single kernel() call runs):
    the last matrix is loaded as 4 quarter DMAs so the tail only waits on a
    256KB transfer + small reduces (-350ns in TimelineSim). y stays on
    SWDGE: 8 separate HWDGE y-writes were modeled ~4us SLOWER — each small
    HBM write pays the ~2us completion-receipt fixed cost, FIFO-serialized
    per ring, while one SWDGE DMA pays it once."""
    f32 = mybir.dt.float32

    acc = accp.tile([128, 128], f32)
    for m in range(BL):
        t = inp.tile([128, 4 * M], f32)
        if m == BL - 1 and tail_split:
            qacc = resp.tile([128, 32], f32)
            xq = x[m].rearrange("(p a) c -> p a c", p=128)
            for q in range(4):
                eng = nc.sync if q % 2 == 0 else nc.scalar
                eng.dma_start(t[:, q * M : (q + 1) * M], xq[:, q, :])
                nc.vector.tensor_reduce(
                    qacc[:, q * 8 : (q + 1) * 8],
                    t[:, q * M : (q + 1) * M].rearrange(
                        "p (g c) -> p g c", c=M // GRID
                    ),
                    axis=mybir.AxisListType.X,
                    op=mybir.AluOpType.min,
                )
            nc.vector.tensor_reduce(
                acc[:, m * 8 : (m + 1) * 8],
                qacc[:].rearrange("p (a g) -> p g a", g=GRID),
                axis=mybir.AxisListType.X,
                op=mybir.AluOpType.min,
            )
        else:
            eng = nc.sync if m % 2 == 0 else nc.scalar
            eng.dma_start(
                t[:],
                x[m].rearrange("(p a) c -> p (a c)", p=128),
            )
            nc.vector.tensor_reduce(
                acc[:, m * 8 : (m + 1) * 8],
                t[:].rearrange("p (a g c) -> p g a c", a=4, c=M // GRID),
                axis=mybir.AxisListType.XY,
                op=mybir.AluOpType.min,
            )

    pt_all = psw.tile([128, 128], f32)
    nc.tensor.transpose(pt_all[:], acc[:], ident[:])
    res = resp.tile([128, 8], f32)
    nc.vector.tensor_reduce(
        res[:],
        pt_all[:].rearrange("p (b qd) -> p b qd", qd=16),
        axis=mybir.AxisListType.X,
        op=mybir.AluOpType.min,
    )
    pt2 = ps2.tile([128, 128], f32)
    nc.tensor.transpose(pt2[0:8, :], res[:], ident[:])
    resT = rtp.tile([128, 128], f32)
    nc.vector.tensor_scalar_add(resT[0:8, :], pt2[0:8, :], 0.0)

    if y_hwdge:
        yv = y[:].rearrange("m (b g) -> b m g", b=8)
        for b in range(GRID):
            eng = nc.sync if b % 2 == 0 else nc.scalar
            eng.dma_start(yv[b], resT[b : b + 1, :])
    else:
        nc.gpsimd.dma_start(
            y[:].rearrange("m (b g) -> b m g", b=8),
            resT[0:8, :].rearrange("b (m g) -> b m g", g=8),
        )


def _make_nc():
    import concourse.bacc as bacc
    import concourse.mybir as mybir

    f32 = mybir.dt.float32
    nc = bacc.Bacc("TRN2", target_bir_lowering=False, debug=False)
    x = nc.dram_tensor("x", [BL, N, M], f32, kind="ExternalInput").ap()
    y = nc.dram_tensor("y", [BL, GRID * GRID], f32, kind="ExternalOutput").ap()
    return nc, mybir, x, y


def _build(n_iters: int = 1, bufs: int = 6, tail_split=True, y_hwdge=False):
    """Python-unrolled build (n_iters=1 is what kernel() runs)."""
    import concourse.tile as tile

    nc, mybir, x, y = _make_nc()
    with tile.TileContext(nc) as tc:
        with (
            tc.tile_pool(name="inp", bufs=bufs) as inp,
            tc.tile_pool(name="accp", bufs=2) as accp,
            tc.tile_pool(name="resp", bufs=2) as resp,
            tc.tile_pool(name="rtp", bufs=2) as rtp,
            tc.tile_pool(name="idp", bufs=1) as idp,
            tc.tile_pool(name="psw", bufs=2, space="PSUM") as psw,
            tc.tile_pool(name="ps2", bufs=2, space="PSUM") as ps2,
        ):
            ident = _emit_prologue(nc, tc, mybir, idp)
            for _ in range(n_iters - 1):
                _emit_iteration(
                    nc, mybir, x, y, ident, inp, accp, resp, rtp, psw, ps2
                )
            _emit_iteration_tail(
                nc, mybir, x, y, ident, inp, accp, resp, rtp, psw, ps2,
                tail_split=tail_split, y_hwdge=y_hwdge,
            )
    nc.compile()
    return nc


def _build_loop(n_bodies: int, unroll: int = 16, bufs: int = 6):
    """Hardware-looped build (for low-noise timing): body = `unroll`
    iterations inside tc.For_i, so device time scales without NEFF bloat.
    unroll=32 + hint_engines=(DVE, SP, Activation) was A/B-measured ~1.3us
    per iteration SLOWER than unroll=16 with no hints — keep 16/no-hints."""
    import concourse.tile as tile

    nc, mybir, x, y = _make_nc()
    with tile.TileContext(nc) as tc:
        with (
            tc.tile_pool(name="inp", bufs=bufs) as inp,
            tc.tile_pool(name="accp", bufs=2) as accp,
            tc.tile_pool(name="resp", bufs=2) as resp,
            tc.tile_pool(name="rtp", bufs=2) as rtp,
            tc.tile_pool(name="idp", bufs=1) as idp,
            tc.tile_pool(name="psw", bufs=2, space="PSUM") as psw,
            tc.tile_pool(name="ps2", bufs=2, space="PSUM") as ps2,
        ):
            ident = _emit_prologue(nc, tc, mybir, idp)
            with tc.For_i(0, n_bodies, 1):
                for _ in range(unroll):
                    _emit_iteration(
                        nc, mybir, x, y, ident, inp, accp, resp, rtp, psw, ps2
                    )
    nc.compile()
    return nc


def _get_nc():
    with _lock:
        if "nc" not in _cache:
            _cache["nc"] = _build()
        return _cache["nc"]


def _get_runner():
    """Build the shard_map-jitted 8-core runner ONCE and reuse it across
    kernel() calls (run_bass_kernel_spmd re-jits per call, ~seconds of host
    overhead). Mirrors bass2jax.run_bass_via_pjrt's multi-core wiring."""
    if "runner" in _cache:
        return _cache["runner"]

    import jax
    from jax.sharding import Mesh, PartitionSpec
    from jax.experimental.shard_map import shard_map

    from concourse import bass2jax, mybir

    nc = _get_nc()
    bass2jax.install_neuronx_cc_hook()

    partition_name = nc.partition_id_tensor.name if nc.partition_id_tensor else None
    in_names, out_names, out_avals = [], [], []
    for alloc in nc.m.functions[0].allocations:
        if not isinstance(alloc, mybir.MemoryLocationSet):
            continue
        name = alloc.memorylocations[0].name
        if alloc.kind == "ExternalInput":
            if name != partition_name:
                in_names.append(name)
        elif alloc.kind == "ExternalOutput":
            out_names.append(name)
            out_avals.append(
                jax.core.ShapedArray(
                    tuple(alloc.tensor_shape), mybir.dt.np(alloc.dtype)
                )
            )
    assert in_names == ["x"] and out_names == ["y"]
    all_in_names = list(in_names) + list(out_names)
    if partition_name is not None:
        all_in_names.append(partition_name)

    def _body(*args):
        operands = list(args)
        if partition_name is not None:
            operands.append(bass2jax.partition_id_tensor())
        outs = bass2jax._bass_exec_p.bind(
            *operands,
            out_avals=tuple(out_avals),
            in_names=tuple(all_in_names),
            out_names=tuple(out_names),
            lowering_input_output_aliases=(),
            sim_require_finite=True,
            sim_require_nnan=True,
            nc=nc,
        )
        return tuple(outs)

    devices = jax.devices()[:NCORES]
    mesh = Mesh(np.asarray(devices), ("core",))
    sharded = jax.jit(
        shard_map(
            _body,
            mesh=mesh,
            in_specs=(PartitionSpec("core"),) * 2,
            out_specs=(PartitionSpec("core"),),
            check_rep=False,
        ),
        donate_argnums=(1,),
        keep_unused=True,
    )
    _cache["runner"] = sharded
    return sharded


def _kernel_fallback(xs: np.ndarray) -> np.ndarray:
    from concourse.bass_utils import run_bass_kernel_spmd

    nc = _get_nc()
    in_maps = [{"x": xs[i * BL : (i + 1) * BL]} for i in range(NCORES)]
    r = run_bass_kernel_spmd(nc, in_maps, list(range(NCORES)))
    return np.concatenate([r.results[i]["y"] for i in range(NCORES)], axis=0)


def kernel(sim_matrices: np.ndarray) -> np.ndarray:
    xs = np.ascontiguousarray(sim_matrices, dtype=np.float32)
    assert xs.shape == (B, N, M), xs.shape
    with _lock:
        try:
            runner = _get_runner()
            zeros = np.zeros((B, GRID * GRID), np.float32)
            (y_global,) = runner(xs, zeros)
            return np.asarray(y_global)
        except Exception:
            return _kernel_fallback(xs)
